# revision 18
# baseline (speedup 1.0000x reference)
"""Trainium2 Bass kernel for block-local (sparse window) attention.

Problem: B=4, S=4096, DIM=768, H=12 heads x DH=64, local window W=256.
    out = (softmax_blocklocal(mask(Q K^T / sqrt(DH))) V) @ Wff + bff

Sharding: 8 cores, core c = (batch c//2, sequence half c%2) -> 2048 tokens
per core = 8 complete 256-token blocks. Projections are per-token, attention
is block-local, FF is per-token => embarrassingly parallel, no collectives.

Per-core kernel (all feature-major to avoid transposes; bf16 matmuls):
  X^T [768,2048] (host-pretransposed bf16)
  Q^T/K^T = lhsT=Wq/Wk [dim,hd] (natural layout), rhs=X^T -> [hd,t]; bias via
    DVE per-partition tensor_scalar add on the PSUM->SBUF copy.
  V = token-major [t,hd]: lhsT=X^T chunk, rhs=Wv; key-padding mask folded in
    via per-partition multiply on the copy (V rows of masked keys zeroed).
  Attention per block is emitted in PE-tiling-mode-coherent phases (mode
  switches drain the PE array, so phases are not interleaved):
    A (64x128 row tiling, tiles T0/T8 concurrent): all 24 score matmuls
      scores^T[k,q], K=64 per head; head parities alternate so pairs pack.
    exp: one ACT op per (head, kc-pairs merged) [128,2,256], scale=1/8.
    B (128x64 col tiling, tiles T0/T1 concurrent): per head pair one
      av tile (attn unnormalized, parities in partition halves) and one
      dp tile = lhsT=mask-replicated [k,64] @ E^T -> denominator REPLICATED
      across the 64 partitions of its head's half (PE does the broadcast).
    One DVE reciprocal + one DVE multiply per pair normalizes both heads.
  out^T[o,t] = lhsT=Wff[hd,o] (natural), rhs=attn^T; bias=bff+bv@Wff (host-
  folded, exact because softmax rows sum to 1) on the ACT copy.
  Emission is software-pipelined in 4 token-quarter units:
    proj(u) -> attn(block 2u, 2u+1) -> FF(u)
  so ACT exp work overlaps PE projection/FF matmuls of neighboring units.
  Host transposes out^T back.
"""

import numpy as np
import ml_dtypes

import concourse.bass as bass
import concourse.mybir as mybir
from concourse import bacc
from concourse.tile import TileContext
from concourse.bass_utils import run_bass_kernel_spmd

B, S, DIM = 4, 4096, 768
H, DH = 12, 64
W = 256
NCORES = 8
T = (B * S) // NCORES       # 2048 tokens per core
NB = T // W                 # 8 blocks per core
NKC = T // 128              # 16 token chunks of 128 per core
DC = DIM // 128             # 6 dim chunks
HC = (H * DH) // 128        # 6 hd chunks
BF16 = mybir.dt.bfloat16
F32 = mybir.dt.float32

_nc_cache = {}


def _build_nc():
    nc = bacc.Bacc()

    # partition-major host layouts: per-partition data contiguous in DRAM so
    # DMA moves few, large elements (xt: 6KB/partition/quarter, w*: 9KB)
    xt_d = nc.declare_dram_parameter("xt", [128, 4, DC, 512], BF16, isOutput=False)
    wq_d = nc.declare_dram_parameter("wq", [128, DC, DIM], BF16, isOutput=False)
    wk_d = nc.declare_dram_parameter("wk", [128, DC, DIM], BF16, isOutput=False)
    wv_d = nc.declare_dram_parameter("wv", [128, DC, DIM], BF16, isOutput=False)
    wff_d = nc.declare_dram_parameter("wff", [128, DC, DIM], BF16, isOutput=False)
    bq_d = nc.declare_dram_parameter("bq", [128, HC], F32, isOutput=False)
    bk_d = nc.declare_dram_parameter("bk", [128, HC], F32, isOutput=False)
    bffe_d = nc.declare_dram_parameter("bffe", [128, DC], F32, isOutput=False)
    # mask as 0/1: per-partition scalar [128, NKC] and 64-wide replicated bf16
    mv_d = nc.declare_dram_parameter("mv", [128, NKC], F32, isOutput=False)
    mbc_d = nc.declare_dram_parameter("mbc", [128, NKC * 64], BF16, isOutput=False)
    out_d = nc.declare_dram_parameter("out", [DIM, T], BF16, isOutput=True)

    Exp = mybir.ActivationFunctionType.Exp
    Ident = mybir.ActivationFunctionType.Identity
    ADD = mybir.AluOpType.add
    MULT = mybir.AluOpType.mult

    with TileContext(nc) as tc:
        with (
            tc.tile_pool(name="const", bufs=1) as cpool,
            tc.tile_pool(name="mm", bufs=2, space="PSUM") as mm_pool,
            tc.tile_pool(name="sps", bufs=2, space="PSUM") as s_pool,
            tc.tile_pool(name="adp", bufs=2, space="PSUM") as ad_pool,
            tc.tile_pool(name="et", bufs=13) as et_pool,
            tc.tile_pool(name="nrm", bufs=4) as nrm_pool,
            tc.tile_pool(name="ob", bufs=3) as ob_pool,
        ):
            # ---- persistent SBUF tensors ----
            xt_sb = cpool.tile([128, 4, DC, 512], BF16, name="xt_sb")
            warm_sb = cpool.tile([128, 512], BF16, name="warm_sb")
            wq_sb = cpool.tile([128, DC, DIM], BF16, name="wq_sb")
            wk_sb = cpool.tile([128, DC, DIM], BF16, name="wk_sb")
            wv_sb = cpool.tile([128, DC, DIM], BF16, name="wv_sb")
            wff_sb = cpool.tile([128, HC, DIM], BF16, name="wff_sb")
            qt_sb = cpool.tile([128, HC, T], BF16, name="qt_sb")
            kt_sb = cpool.tile([128, HC, T], BF16, name="kt_sb")
            v_sb = cpool.tile([128, NKC, DIM], BF16, name="v_sb")
            at_sb = cpool.tile([128, HC, T], BF16, name="at_sb")
            bq_sb = cpool.tile([128, HC], F32, name="bq_sb")
            bk_sb = cpool.tile([128, HC], F32, name="bk_sb")
            bffe_sb = cpool.tile([128, DC], F32, name="bffe_sb")
            mv_sb = cpool.tile([128, NKC], F32, name="mv_sb")
            mbc_sb = cpool.tile([128, NKC, 64], BF16, name="mbc_sb")

            # ---- load inputs: batched descriptors (one per tensor slice),
            # split across the two HW DGE queues (sync=SP, scalar=ACT; the
            # ACT queue is idle during the load phase). First Q-proj group
            # needs bq + wq[hc0] + xt quarter 0 only.
            xt_v = xt_d.ap()
            wq_v = wq_d.ap()
            wk_v = wk_d.ap()
            wv_v = wv_d.ap()
            wff_v = wff_d.ap()
            # PE warm-up: ~4us of dummy matmuls on a memset tile during the
            # DMA-load head, so HAM reaches K=8/8 before real matmuls start
            # (else the first ~3.4us of real MMs run at 1.2 GHz).
            nc.gpsimd.memset(warm_sb[:], 0.0)
            warm_ps = mm_pool.tile([128, 512], F32, tag="mm", name="warm_ps")
            for _ in range(10):
                nc.tensor.matmul(
                    warm_ps[:], warm_sb[:, 0:128], warm_sb[:], start=True, stop=True
                )

            nc.sync.dma_start(out=bq_sb[:], in_=bq_d.ap())
            nc.scalar.dma_start(out=wq_sb[:, :, 0:128], in_=wq_v[:, :, 0:128])
            # xt quarter 0 per-dc so the first accumulation group's MMs
            # unlock chunk-by-chunk as transfers land
            for dc in range(DC):
                nc.sync.dma_start(out=xt_sb[:, 0, dc], in_=xt_v[:, 0, dc])
            nc.scalar.dma_start(out=wq_sb[:, :, 128:384], in_=wq_v[:, :, 128:384])
            nc.scalar.dma_start(out=wq_sb[:, :, 384:768], in_=wq_v[:, :, 384:768])
            nc.sync.dma_start(out=bk_sb[:], in_=bk_d.ap())
            nc.sync.dma_start(out=mv_sb[:], in_=mv_d.ap())
            nc.scalar.dma_start(out=wk_sb[:], in_=wk_v[:])
            nc.sync.dma_start(out=wv_sb[:], in_=wv_v[:])
            nc.scalar.dma_start(out=xt_sb[:, 1], in_=xt_v[:, 1])
            # needed only from the attention/FF stages (~25us+)
            nc.sync.dma_start(
                out=mbc_sb[:], in_=mbc_d.ap().rearrange("p (c o) -> p c o", o=64)
            )
            nc.sync.dma_start(out=bffe_sb[:], in_=bffe_d.ap())
            nc.scalar.dma_start(out=xt_sb[:, 2], in_=xt_v[:, 2])
            nc.sync.dma_start(out=xt_sb[:, 3], in_=xt_v[:, 3])
            nc.scalar.dma_start(out=wff_sb[:], in_=wff_v[:])

            def proj_qk(w_sb, b_sb, o_sb, tt):
                # one token-quarter of a Q^T/K^T projection: out [hd, 512]
                for hc in range(HC):
                    ps = mm_pool.tile([128, 512], F32, tag="mm", name="ps")
                    for dc in range(DC):
                        nc.tensor.matmul(
                            ps[:],
                            w_sb[:, dc, hc * 128:(hc + 1) * 128],
                            xt_sb[:, tt, dc],
                            start=(dc == 0),
                            stop=(dc == DC - 1),
                        )
                    nc.vector.tensor_scalar(
                        out=o_sb[:, hc, tt * 512:(tt + 1) * 512],
                        in0=ps[:],
                        scalar1=b_sb[:, hc:hc + 1],
                        scalar2=None,
                        op0=ADD,
                    )

            def proj_v(kc):
                # V token-chunk [128 tokens, 768], mask folded in.
                # dc outer / half inner so consecutive matmuls share lhsT.
                tt, c0 = divmod(kc, 4)
                c0 *= 128
                ps = [
                    mm_pool.tile([128, 384], F32, tag="mm", name="ps"),
                    mm_pool.tile([128, 384], F32, tag="mm", name="ps"),
                ]
                for dc in range(DC):
                    for half in range(2):
                        nc.tensor.matmul(
                            ps[half][:],
                            xt_sb[:, tt, dc, c0:c0 + 128],
                            wv_sb[:, dc, half * 384:(half + 1) * 384],
                            start=(dc == 0),
                            stop=(dc == DC - 1),
                        )
                for half in range(2):
                    nc.vector.tensor_scalar(
                        out=v_sb[:, kc, half * 384:(half + 1) * 384],
                        in0=ps[half][:],
                        scalar1=mv_sb[:, kc:kc + 1],
                        scalar2=None,
                        op0=MULT,
                    )

            # ets_by_block[blk] -> list of 6 merged exp tiles [128, 2par,
            # 2kc, 256] bf16
            ets_by_block = {}

            def scores_exp(blk):
                # phase A: scores (row-tiled 64x128, T0/T8 par-pairs pack),
                # both parities of a head pair in one 2-bank PSUM tile, then
                # ONE exp ACTIVATE per head pair over [128, 1024].
                q0 = blk * 256
                ets = []
                for hp in range(H // 2):
                    sp = s_pool.tile([128, 2, 2, 256], F32, tag="s", name="sp")
                    for kc in range(2):
                        k0 = q0 + kc * 128
                        for par in range(2):
                            hr = par * 64
                            nc.tensor.matmul(
                                sp[:, par, kc],
                                kt_sb[hr:hr + 64, hp, k0:k0 + 128],
                                qt_sb[hr:hr + 64, hp, q0:q0 + 256],
                                start=True, stop=True,
                            )
                    et = et_pool.tile([128, 2, 2, 256], BF16, tag="et", name="et")
                    nc.scalar.activation(et[:], sp[:], Exp, bias=0.0, scale=0.125)
                    ets.append(et)
                ets_by_block[blk] = ets

            def avdp(blk):
                # phase B: av (cols 0:256) + denominator (cols 256:512) in
                # one bank, col-tiled 128x64 T0/T1; kc outer / par inner so
                # the (par0, par1) col-tile pairs issue back-to-back and
                # execute concurrently. Emitted one block behind scores_exp
                # so every exp tile is ready -> solid col-mode burst.
                q0 = blk * 256
                ets = ets_by_block.pop(blk)
                for hp in range(H // 2):
                    et = ets[hp]
                    ad = ad_pool.tile([128, 512], F32, tag="ad", name="ad")
                    for kc in range(2):
                        tkc = blk * 2 + kc
                        for par in range(2):
                            hr = par * 64
                            h = 2 * hp + par
                            nc.tensor.matmul(
                                ad[hr:hr + 64, 0:256],
                                v_sb[:, tkc, h * 64:(h + 1) * 64],
                                et[:, par, kc],
                                start=(kc == 0), stop=(kc == 1),
                            )
                    for kc in range(2):
                        tkc = blk * 2 + kc
                        for par in range(2):
                            hr = par * 64
                            nc.tensor.matmul(
                                ad[hr:hr + 64, 256:512],
                                mbc_sb[:, tkc],
                                et[:, par, kc],
                                start=(kc == 0), stop=(kc == 1),
                            )
                    rc = nrm_pool.tile([128, 256], F32, tag="rc", name="rc")
                    nc.vector.reciprocal_approx_fast(rc[:], ad[:, 256:512])
                    nc.vector.tensor_mul(
                        at_sb[:, hp, q0:q0 + 256], ad[:, 0:256], rc[:]
                    )

            def ff(tt):
                for oc in range(DC):
                    ps = mm_pool.tile([128, 512], F32, tag="mm", name="ps")
                    for hc in range(HC):
                        nc.tensor.matmul(
                            ps[:],
                            wff_sb[:, hc, oc * 128:(oc + 1) * 128],
                            at_sb[:, hc, tt * 512:(tt + 1) * 512],
                            start=(hc == 0),
                            stop=(hc == HC - 1),
                        )
                    ob = ob_pool.tile([128, 512], BF16, tag="ob", name="ob")
                    nc.scalar.activation(
                        ob[:], ps[:], Ident, bias=bffe_sb[:, oc:oc + 1], scale=1.0
                    )
                    nc.sync.dma_start(
                        out=out_d.ap()[oc * 128:(oc + 1) * 128,
                                       tt * 512:(tt + 1) * 512],
                        in_=ob[:],
                    )

            # ---- emission: block-pipelined attention. scores_exp(b) runs
            # one block ahead of avdp(b-1) so avdp never waits on ACT and
            # each phase runs as a solid same-tiling-mode burst. ff(u-1)
            # lands between projections and attention as PE filler during
            # the ACT-bound stretches.
            for u in range(4):
                proj_qk(wq_sb, bq_sb, qt_sb, u)
                proj_qk(wk_sb, bk_sb, kt_sb, u)
                for kc in range(4 * u, 4 * u + 4):
                    proj_v(kc)
                if u == 0:
                    scores_exp(0)
                    scores_exp(1)
                    avdp(0)
                else:
                    scores_exp(2 * u)
                    avdp(2 * u - 1)
                    scores_exp(2 * u + 1)
                    avdp(2 * u)
                    # ff(u-1) emitted after attention: ready-but-lower-
                    # priority PE filler for the ACT-bound stretches, and
                    # its ACT bias copies queue behind this quarter's exps
                    ff(u - 1)
            avdp(7)
            ff(3)

    nc.finalize()
    return nc


def _get_nc():
    if "nc" not in _nc_cache:
        _nc_cache["nc"] = _build_nc()
    return _nc_cache["nc"]


def _prep_in_maps(X, mask, Wq, bq, Wk, bk, Wv, bv, Wff, bff):
    bf = ml_dtypes.bfloat16

    def wprep(w):  # [DIM, DIM] -> partition-major [128, DC, DIM]
        return np.ascontiguousarray(
            w.astype(bf).reshape(DC, 128, DIM).transpose(1, 0, 2))

    wq_b, wk_b, wv_b, wff_b = wprep(Wq), wprep(Wk), wprep(Wv), wprep(Wff)
    # per-partition bias layouts: [128, nchunks] with col = chunk
    bq_t = np.ascontiguousarray(bq.astype(np.float32).reshape(HC, 128).T)
    bk_t = np.ascontiguousarray(bk.astype(np.float32).reshape(HC, 128).T)
    bffe = (bff.astype(np.float64)
            + bv.astype(np.float64) @ Wff.astype(np.float64)).astype(np.float32)
    bffe_t = np.ascontiguousarray(bffe.reshape(DC, 128).T)

    in_maps = []
    for c in range(NCORES):
        b, s0 = divmod(c, 2)
        s0 *= T
        # partition-major, quarter-major X^T: [128, 4, DC, 512]
        xt = np.ascontiguousarray(
            X[b, s0:s0 + T, :].astype(bf)
            .reshape(4, 512, DC, 128).transpose(3, 0, 2, 1))
        mvalid = (mask[b, s0:s0 + T] > 0).astype(np.float32)  # [T] 0/1
        mv_t = np.ascontiguousarray(mvalid.reshape(NKC, 128).T)  # [128, NKC]
        mbc = np.ascontiguousarray(
            np.broadcast_to(mv_t[:, :, None], (128, NKC, 64))
            .reshape(128, NKC * 64).astype(bf))
        in_maps.append({
            "xt": xt, "wq": wq_b, "wk": wk_b, "wv": wv_b, "wff": wff_b,
            "bq": bq_t, "bk": bk_t, "bffe": bffe_t,
            "mv": mv_t, "mbc": mbc,
        })
    return in_maps


def _assemble(results):
    out = np.empty((B, S, DIM), np.float32)
    for c in range(NCORES):
        b, s0 = divmod(c, 2)
        s0 *= T
        out[b, s0:s0 + T, :] = results[c]["out"].T.astype(np.float32)
    return out


def run(trace=False, **inputs):
    nc = _get_nc()
    in_maps = _prep_in_maps(**inputs)
    res = run_bass_kernel_spmd(
        nc, in_maps, core_ids=list(range(NCORES)), trace=trace
    )
    return _assemble(res.results), res


def kernel(**inputs) -> np.ndarray:
    out, _ = run(trace=False, **inputs)
    return out



# revision 27
# speedup vs baseline: 1.2063x; 1.2063x over previous
"""Trainium2 Bass kernel for block-local (sparse window) attention.

Problem: B=4, S=4096, DIM=768, H=12 heads x DH=64, local window W=256.
    out = (softmax_blocklocal(mask(Q K^T / sqrt(DH))) V) @ Wff + bff

Sharding: 8 cores, core c = (batch c//2, sequence half c%2) -> 2048 tokens
per core = 8 complete 256-token blocks. Projections are per-token, attention
is block-local, FF is per-token => embarrassingly parallel, no collectives.

Per-core kernel (all feature-major to avoid transposes; bf16 matmuls):
  X^T (host-prepped partition-major [128, quarter, dc, 512] bf16)
  Q^T/K^T = lhsT=Wq/Wk chunks (hc-major host layout), rhs=X^T -> [hd,t];
    bias via per-partition add on the PSUM->SBUF copy, alternating ACT/DVE.
  V = token-major [t,hd]: lhsT=X^T chunk, rhs=Wv; key-padding mask folded in
    via per-partition multiply on the copy (V rows of masked keys zeroed).
  Attention, block-software-pipelined (scores of block b+1 are emitted
  before av/dp of block b so av/dp never stalls on ACT and each phase runs
  as a solid same-tiling-mode burst):
    scores (64x128 row tiling, parity pairs T0/T8 concurrent) into a
      2-bank PSUM tile [128, par, kc, 256] per head pair, then ONE exp
      ACTIVATE [128,1024] per head pair (scale=1/8) -> bf16 et tile.
    av+den (128x64 col tiling, kc outer / par inner so T0/T1 pairs issue
      back-to-back): av in bank cols 0:256, denominator (lhsT = mask
      replicated 64-wide; PE broadcasts across partitions) in 256:512.
    One DVE reciprocal + multiply per head pair normalizes both heads.
  out^T[o,t] = lhsT=Wff, rhs=attn^T; bias=bff+bv@Wff (host-folded, exact
  because softmax rows sum to 1) on the ACT copy; bf16 output DMA.
  Emission per quarter u: Q,K,V proj -> S(2u), A(2u-1), S(2u+1), A(2u) ->
  ff(u-1) (ready-but-lower-priority PE filler for ACT-bound stretches).
  ~13 warmup matmuls on a memset tile bridge the DMA-load head so HAM
  reaches K=8/8 before real matmuls start; input DMA uses fat partition-
  major descriptors split across both HW queues (sync + scalar).
  Host converts the bf16 out^T back to [B,S,DIM] fp32.
"""

import numpy as np
import ml_dtypes

import concourse.bass as bass
import concourse.mybir as mybir
from concourse import bacc
from concourse.tile import TileContext
from concourse.bass_utils import run_bass_kernel_spmd

B, S, DIM = 4, 4096, 768
H, DH = 12, 64
W = 256
NCORES = 8
T = (B * S) // NCORES       # 2048 tokens per core
NB = T // W                 # 8 blocks per core
NKC = T // 128              # 16 token chunks of 128 per core
DC = DIM // 128             # 6 dim chunks
HC = (H * DH) // 128        # 6 hd chunks
BF16 = mybir.dt.bfloat16
F32 = mybir.dt.float32

_nc_cache = {}


def _build_nc():
    nc = bacc.Bacc()

    # partition-major host layouts: per-partition data contiguous in DRAM so
    # DMA moves few, large elements (xt: 6KB/partition/quarter, w*: 9KB)
    xt_d = nc.declare_dram_parameter("xt", [128, 4, DC, 512], BF16, isOutput=False)
    # wq/wk are hc-major so the first head-chunk is one small fast descriptor
    wq_d = nc.declare_dram_parameter("wq", [128, HC, DC, 128], BF16, isOutput=False)
    wk_d = nc.declare_dram_parameter("wk", [128, HC, DC, 128], BF16, isOutput=False)
    wv_d = nc.declare_dram_parameter("wv", [128, DC, DIM], BF16, isOutput=False)
    wff_d = nc.declare_dram_parameter("wff", [128, DC, DIM], BF16, isOutput=False)
    bq_d = nc.declare_dram_parameter("bq", [128, HC], F32, isOutput=False)
    bk_d = nc.declare_dram_parameter("bk", [128, HC], F32, isOutput=False)
    bffe_d = nc.declare_dram_parameter("bffe", [128, DC], F32, isOutput=False)
    # mask as 0/1: per-partition scalar [128, NKC] and 64-wide replicated bf16
    mv_d = nc.declare_dram_parameter("mv", [128, NKC], F32, isOutput=False)
    mbc_d = nc.declare_dram_parameter("mbc", [128, NKC * 64], BF16, isOutput=False)
    out_d = nc.declare_dram_parameter("out", [DIM, T], BF16, isOutput=True)

    Exp = mybir.ActivationFunctionType.Exp
    Ident = mybir.ActivationFunctionType.Identity
    ADD = mybir.AluOpType.add
    MULT = mybir.AluOpType.mult

    with TileContext(nc) as tc:
        with (
            tc.tile_pool(name="const", bufs=1) as cpool,
            tc.tile_pool(name="mm", bufs=2, space="PSUM") as mm_pool,
            tc.tile_pool(name="sps", bufs=2, space="PSUM") as s_pool,
            tc.tile_pool(name="adp", bufs=2, space="PSUM") as ad_pool,
            tc.tile_pool(name="et", bufs=13) as et_pool,
            tc.tile_pool(name="nrm", bufs=4) as nrm_pool,
            tc.tile_pool(name="ob", bufs=3) as ob_pool,
        ):
            # ---- persistent SBUF tensors ----
            xt_sb = cpool.tile([128, 4, DC, 512], BF16, name="xt_sb")
            warm_sb = cpool.tile([128, 512], BF16, name="warm_sb")
            wq_sb = cpool.tile([128, HC, DC, 128], BF16, name="wq_sb")
            wk_sb = cpool.tile([128, HC, DC, 128], BF16, name="wk_sb")
            wv_sb = cpool.tile([128, DC, DIM], BF16, name="wv_sb")
            wff_sb = cpool.tile([128, HC, DIM], BF16, name="wff_sb")
            qt_sb = cpool.tile([128, HC, T], BF16, name="qt_sb")
            kt_sb = cpool.tile([128, HC, T], BF16, name="kt_sb")
            v_sb = cpool.tile([128, NKC, DIM], BF16, name="v_sb")
            at_sb = cpool.tile([128, HC, T], BF16, name="at_sb")
            bq_sb = cpool.tile([128, HC], F32, name="bq_sb")
            bk_sb = cpool.tile([128, HC], F32, name="bk_sb")
            bffe_sb = cpool.tile([128, DC], F32, name="bffe_sb")
            mv_sb = cpool.tile([128, NKC], F32, name="mv_sb")
            mbc_sb = cpool.tile([128, NKC, 64], BF16, name="mbc_sb")

            # ---- load inputs: batched descriptors (one per tensor slice),
            # split across the two HW DGE queues (sync=SP, scalar=ACT; the
            # ACT queue is idle during the load phase). First Q-proj group
            # needs bq + wq[hc0] + xt quarter 0 only.
            xt_v = xt_d.ap()
            wq_v = wq_d.ap()
            wk_v = wk_d.ap()
            wv_v = wv_d.ap()
            wff_v = wff_d.ap()
            # PE warm-up: ~4us of dummy matmuls on a memset tile during the
            # DMA-load head, so HAM reaches K=8/8 before real matmuls start
            # (else the first ~3.4us of real MMs run at 1.2 GHz).
            nc.gpsimd.memset(warm_sb[:], 0.0)
            warm_ps = mm_pool.tile([128, 512], F32, tag="mm", name="warm_ps")
            for _ in range(13):
                nc.tensor.matmul(
                    warm_ps[:], warm_sb[:, 0:128], warm_sb[:], start=True, stop=True
                )

            # xt-q0 first as one fat descriptor (6KB/partition elements) on
            # sync; wq per-hc on scalar. Both land ~13us, right as the 13
            # warmup matmuls end -> dense warm start, no HAM re-throttle.
            nc.sync.dma_start(out=xt_sb[:, 0], in_=xt_v[:, 0])
            nc.scalar.dma_start(out=wq_sb[:, 0], in_=wq_v[:, 0])
            nc.scalar.dma_start(out=wq_sb[:, 1], in_=wq_v[:, 1])
            nc.sync.dma_start(out=bq_sb[:], in_=bq_d.ap())
            nc.scalar.dma_start(out=wq_sb[:, 2:HC], in_=wq_v[:, 2:HC])
            nc.sync.dma_start(out=bk_sb[:], in_=bk_d.ap())
            nc.sync.dma_start(out=mv_sb[:], in_=mv_d.ap())
            nc.scalar.dma_start(out=wk_sb[:], in_=wk_v[:])
            nc.sync.dma_start(out=wv_sb[:], in_=wv_v[:])
            nc.scalar.dma_start(out=xt_sb[:, 1], in_=xt_v[:, 1])
            # needed only from the attention/FF stages (~25us+)
            nc.sync.dma_start(
                out=mbc_sb[:], in_=mbc_d.ap().rearrange("p (c o) -> p c o", o=64)
            )
            nc.sync.dma_start(out=bffe_sb[:], in_=bffe_d.ap())
            nc.scalar.dma_start(out=xt_sb[:, 2], in_=xt_v[:, 2])
            nc.sync.dma_start(out=xt_sb[:, 3], in_=xt_v[:, 3])
            nc.scalar.dma_start(out=wff_sb[:], in_=wff_v[:])

            def proj_qk(w_sb, b_sb, o_sb, tt):
                # one token-quarter of a Q^T/K^T projection: out [hd, 512].
                # PSUM->SBUF bias copies alternate DVE/ACT so neither queue
                # gates the mm_pool slot recycle during projection phases.
                for hc in range(HC):
                    ps = mm_pool.tile([128, 512], F32, tag="mm", name="ps")
                    for dc in range(DC):
                        nc.tensor.matmul(
                            ps[:],
                            w_sb[:, hc, dc],
                            xt_sb[:, tt, dc],
                            start=(dc == 0),
                            stop=(dc == DC - 1),
                        )
                    if hc % 2 == 0:
                        nc.scalar.activation(
                            o_sb[:, hc, tt * 512:(tt + 1) * 512], ps[:],
                            Ident, bias=b_sb[:, hc:hc + 1], scale=1.0,
                        )
                    else:
                        nc.vector.tensor_scalar(
                            out=o_sb[:, hc, tt * 512:(tt + 1) * 512],
                            in0=ps[:],
                            scalar1=b_sb[:, hc:hc + 1],
                            scalar2=None,
                            op0=ADD,
                        )

            def proj_v(kc):
                # V token-chunk [128 tokens, 768], mask folded in.
                # dc outer / half inner so consecutive matmuls share lhsT.
                tt, c0 = divmod(kc, 4)
                c0 *= 128
                ps = [
                    mm_pool.tile([128, 384], F32, tag="mm", name="ps"),
                    mm_pool.tile([128, 384], F32, tag="mm", name="ps"),
                ]
                for dc in range(DC):
                    for half in range(2):
                        nc.tensor.matmul(
                            ps[half][:],
                            xt_sb[:, tt, dc, c0:c0 + 128],
                            wv_sb[:, dc, half * 384:(half + 1) * 384],
                            start=(dc == 0),
                            stop=(dc == DC - 1),
                        )
                for half in range(2):
                    nc.vector.tensor_scalar(
                        out=v_sb[:, kc, half * 384:(half + 1) * 384],
                        in0=ps[half][:],
                        scalar1=mv_sb[:, kc:kc + 1],
                        scalar2=None,
                        op0=MULT,
                    )

            # ets_by_block[blk] -> list of 6 merged exp tiles [128, 2par,
            # 2kc, 256] bf16
            ets_by_block = {}

            def scores_exp(blk):
                # phase A: scores (row-tiled 64x128, T0/T8 par-pairs pack),
                # both parities of a head pair in one 2-bank PSUM tile, then
                # ONE exp ACTIVATE per head pair over [128, 1024].
                q0 = blk * 256
                ets = []
                for hp in range(H // 2):
                    sp = s_pool.tile([128, 2, 2, 256], F32, tag="s", name="sp")
                    for kc in range(2):
                        k0 = q0 + kc * 128
                        for par in range(2):
                            hr = par * 64
                            nc.tensor.matmul(
                                sp[:, par, kc],
                                kt_sb[hr:hr + 64, hp, k0:k0 + 128],
                                qt_sb[hr:hr + 64, hp, q0:q0 + 256],
                                start=True, stop=True,
                            )
                    et = et_pool.tile([128, 2, 2, 256], BF16, tag="et", name="et")
                    nc.scalar.activation(et[:], sp[:], Exp, bias=0.0, scale=0.125)
                    ets.append(et)
                ets_by_block[blk] = ets

            def avdp(blk):
                # phase B: av (cols 0:256) + denominator (cols 256:512) in
                # one bank, col-tiled 128x64 T0/T1; kc outer / par inner so
                # the (par0, par1) col-tile pairs issue back-to-back and
                # execute concurrently. Emitted one block behind scores_exp
                # so every exp tile is ready -> solid col-mode burst.
                q0 = blk * 256
                ets = ets_by_block.pop(blk)
                for hp in range(H // 2):
                    et = ets[hp]
                    ad = ad_pool.tile([128, 512], F32, tag="ad", name="ad")
                    for kc in range(2):
                        tkc = blk * 2 + kc
                        for par in range(2):
                            hr = par * 64
                            h = 2 * hp + par
                            nc.tensor.matmul(
                                ad[hr:hr + 64, 0:256],
                                v_sb[:, tkc, h * 64:(h + 1) * 64],
                                et[:, par, kc],
                                start=(kc == 0), stop=(kc == 1),
                            )
                    for kc in range(2):
                        tkc = blk * 2 + kc
                        for par in range(2):
                            hr = par * 64
                            nc.tensor.matmul(
                                ad[hr:hr + 64, 256:512],
                                mbc_sb[:, tkc],
                                et[:, par, kc],
                                start=(kc == 0), stop=(kc == 1),
                            )
                    rc = nrm_pool.tile([128, 256], F32, tag="rc", name="rc")
                    nc.vector.reciprocal_approx_fast(rc[:], ad[:, 256:512])
                    nc.vector.tensor_mul(
                        at_sb[:, hp, q0:q0 + 256], ad[:, 0:256], rc[:]
                    )

            def ff(tt):
                for oc in range(DC):
                    ps = mm_pool.tile([128, 512], F32, tag="mm", name="ps")
                    for hc in range(HC):
                        nc.tensor.matmul(
                            ps[:],
                            wff_sb[:, hc, oc * 128:(oc + 1) * 128],
                            at_sb[:, hc, tt * 512:(tt + 1) * 512],
                            start=(hc == 0),
                            stop=(hc == HC - 1),
                        )
                    ob = ob_pool.tile([128, 512], BF16, tag="ob", name="ob")
                    nc.scalar.activation(
                        ob[:], ps[:], Ident, bias=bffe_sb[:, oc:oc + 1], scale=1.0
                    )
                    nc.sync.dma_start(
                        out=out_d.ap()[oc * 128:(oc + 1) * 128,
                                       tt * 512:(tt + 1) * 512],
                        in_=ob[:],
                    )

            # ---- emission: block-pipelined attention. scores_exp(b) runs
            # one block ahead of avdp(b-1) so avdp never waits on ACT and
            # each phase runs as a solid same-tiling-mode burst. ff(u-1)
            # lands between projections and attention as PE filler during
            # the ACT-bound stretches.
            for u in range(4):
                proj_qk(wq_sb, bq_sb, qt_sb, u)
                proj_qk(wk_sb, bk_sb, kt_sb, u)
                for kc in range(4 * u, 4 * u + 4):
                    proj_v(kc)
                if u == 0:
                    scores_exp(0)
                    scores_exp(1)
                    avdp(0)
                else:
                    scores_exp(2 * u)
                    avdp(2 * u - 1)
                    scores_exp(2 * u + 1)
                    avdp(2 * u)
                    # ff(u-1) emitted after attention: ready-but-lower-
                    # priority PE filler for the ACT-bound stretches, and
                    # its ACT bias copies queue behind this quarter's exps
                    ff(u - 1)
            avdp(7)
            ff(3)

    nc.finalize()
    return nc


def _get_nc():
    if "nc" not in _nc_cache:
        _nc_cache["nc"] = _build_nc()
    return _nc_cache["nc"]


def _prep_in_maps(X, mask, Wq, bq, Wk, bk, Wv, bv, Wff, bff):
    bf = ml_dtypes.bfloat16

    def wprep(w):  # [DIM, DIM] -> partition-major [128, DC, DIM]
        return np.ascontiguousarray(
            w.astype(bf).reshape(DC, 128, DIM).transpose(1, 0, 2))

    def wprep_hc(w):  # [DIM, DIM] -> hc-major [128, HC, DC, 128]
        return np.ascontiguousarray(
            w.astype(bf).reshape(DC, 128, HC, 128).transpose(1, 2, 0, 3))

    wq_b, wk_b = wprep_hc(Wq), wprep_hc(Wk)
    wv_b, wff_b = wprep(Wv), wprep(Wff)
    # per-partition bias layouts: [128, nchunks] with col = chunk
    bq_t = np.ascontiguousarray(bq.astype(np.float32).reshape(HC, 128).T)
    bk_t = np.ascontiguousarray(bk.astype(np.float32).reshape(HC, 128).T)
    bffe = (bff.astype(np.float64)
            + bv.astype(np.float64) @ Wff.astype(np.float64)).astype(np.float32)
    bffe_t = np.ascontiguousarray(bffe.reshape(DC, 128).T)

    in_maps = []
    for c in range(NCORES):
        b, s0 = divmod(c, 2)
        s0 *= T
        # partition-major, quarter-major X^T: [128, 4, DC, 512]
        xt = np.ascontiguousarray(
            X[b, s0:s0 + T, :].astype(bf)
            .reshape(4, 512, DC, 128).transpose(3, 0, 2, 1))
        mvalid = (mask[b, s0:s0 + T] > 0).astype(np.float32)  # [T] 0/1
        mv_t = np.ascontiguousarray(mvalid.reshape(NKC, 128).T)  # [128, NKC]
        mbc = np.ascontiguousarray(
            np.broadcast_to(mv_t[:, :, None], (128, NKC, 64))
            .reshape(128, NKC * 64).astype(bf))
        in_maps.append({
            "xt": xt, "wq": wq_b, "wk": wk_b, "wv": wv_b, "wff": wff_b,
            "bq": bq_t, "bk": bk_t, "bffe": bffe_t,
            "mv": mv_t, "mbc": mbc,
        })
    return in_maps


def _assemble(results):
    out = np.empty((B, S, DIM), np.float32)
    for c in range(NCORES):
        b, s0 = divmod(c, 2)
        s0 *= T
        out[b, s0:s0 + T, :] = results[c]["out"].T.astype(np.float32)
    return out


def run(trace=False, **inputs):
    nc = _get_nc()
    in_maps = _prep_in_maps(**inputs)
    res = run_bass_kernel_spmd(
        nc, in_maps, core_ids=list(range(NCORES)), trace=trace
    )
    return _assemble(res.results), res


def kernel(**inputs) -> np.ndarray:
    out, _ = run(trace=False, **inputs)
    return out



# revision 31
# speedup vs baseline: 1.2121x; 1.0048x over previous
"""Trainium2 Bass kernel for block-local (sparse window) attention.

Problem: B=4, S=4096, DIM=768, H=12 heads x DH=64, local window W=256.
    out = (softmax_blocklocal(mask(Q K^T / sqrt(DH))) V) @ Wff + bff

Sharding: 8 cores, core c = (batch c//2, sequence half c%2) -> 2048 tokens
per core = 8 complete 256-token blocks. Projections are per-token, attention
is block-local, FF is per-token => embarrassingly parallel, no collectives.

Per-core kernel (all feature-major to avoid transposes; bf16 matmuls):
  X^T (host-prepped partition-major [128, quarter, dc, 512] bf16)
  Q^T/K^T = lhsT=Wq/Wk chunks (hc-major host layout), rhs=X^T -> [hd,t];
    bias via per-partition add on the PSUM->SBUF copy, alternating ACT/DVE.
  V = token-major [t,hd]: lhsT=X^T chunk, rhs=Wv; key-padding mask folded in
    via per-partition multiply on the copy (V rows of masked keys zeroed).
  Attention, block-software-pipelined (scores of block b+1 are emitted
  before av/dp of block b so av/dp never stalls on ACT and each phase runs
  as a solid same-tiling-mode burst):
    scores (64x128 row tiling, parity pairs T0/T8 concurrent) into a
      2-bank PSUM tile [128, par, kc, 256] per head pair, then ONE exp
      ACTIVATE [128,1024] per head pair (scale=1/8) -> bf16 et tile.
    av+den (128x64 col tiling, kc outer / par inner so T0/T1 pairs issue
      back-to-back): av in bank cols 0:256, denominator (lhsT = mask
      replicated 64-wide; PE broadcasts across partitions) in 256:512.
    One DVE reciprocal + multiply per head pair normalizes both heads.
  out^T[o,t] = lhsT=Wff, rhs=attn^T; bias=bff+bv@Wff (host-folded, exact
  because softmax rows sum to 1) on the ACT copy; bf16 output DMA.
  Emission per quarter u: Q,K,V proj -> S(2u), A(2u-1), S(2u+1), A(2u) ->
  ff(u-1) (ready-but-lower-priority PE filler for ACT-bound stretches).
  ~13 warmup matmuls on a memset tile bridge the DMA-load head so HAM
  reaches K=8/8 before real matmuls start; input DMA uses fat partition-
  major descriptors split across both HW queues (sync + scalar).
  Host converts the bf16 out^T back to [B,S,DIM] fp32.
"""

import numpy as np
import ml_dtypes

import concourse.bass as bass
import concourse.mybir as mybir
from concourse import bacc
from concourse.tile import TileContext
from concourse.bass_utils import run_bass_kernel_spmd

B, S, DIM = 4, 4096, 768
H, DH = 12, 64
W = 256
NCORES = 8
T = (B * S) // NCORES       # 2048 tokens per core
NB = T // W                 # 8 blocks per core
NKC = T // 128              # 16 token chunks of 128 per core
DC = DIM // 128             # 6 dim chunks
HC = (H * DH) // 128        # 6 hd chunks
BF16 = mybir.dt.bfloat16
F32 = mybir.dt.float32

_nc_cache = {}


def _build_nc():
    nc = bacc.Bacc()

    # partition-major host layouts: per-partition data contiguous in DRAM so
    # DMA moves few, large elements (xt: 6KB/partition/quarter, w*: 9KB)
    xt_d = nc.declare_dram_parameter("xt", [128, 4, DC, 512], BF16, isOutput=False)
    # wq/wk are hc-major so the first head-chunk is one small fast descriptor
    wq_d = nc.declare_dram_parameter("wq", [128, HC, DC, 128], BF16, isOutput=False)
    wk_d = nc.declare_dram_parameter("wk", [128, HC, DC, 128], BF16, isOutput=False)
    wv_d = nc.declare_dram_parameter("wv", [128, DC, DIM], BF16, isOutput=False)
    wff_d = nc.declare_dram_parameter("wff", [128, DC, DIM], BF16, isOutput=False)
    bq_d = nc.declare_dram_parameter("bq", [128, HC], F32, isOutput=False)
    bk_d = nc.declare_dram_parameter("bk", [128, HC], F32, isOutput=False)
    bffe_d = nc.declare_dram_parameter("bffe", [128, DC], F32, isOutput=False)
    # mask as 0/1: per-partition scalar [128, NKC] and 64-wide replicated bf16
    mv_d = nc.declare_dram_parameter("mv", [128, NKC], F32, isOutput=False)
    mbc_d = nc.declare_dram_parameter("mbc", [128, NKC * 64], BF16, isOutput=False)
    out_d = nc.declare_dram_parameter("out", [DIM, T], BF16, isOutput=True)

    Exp = mybir.ActivationFunctionType.Exp
    Ident = mybir.ActivationFunctionType.Identity
    ADD = mybir.AluOpType.add
    MULT = mybir.AluOpType.mult

    with TileContext(nc) as tc:
        with (
            tc.tile_pool(name="const", bufs=1) as cpool,
            tc.tile_pool(name="mm", bufs=2, space="PSUM") as mm_pool,
            tc.tile_pool(name="sps", bufs=2, space="PSUM") as s_pool,
            tc.tile_pool(name="adp", bufs=2, space="PSUM") as ad_pool,
            tc.tile_pool(name="et", bufs=13) as et_pool,
            tc.tile_pool(name="nrm", bufs=4) as nrm_pool,
            tc.tile_pool(name="ob", bufs=3) as ob_pool,
        ):
            # ---- persistent SBUF tensors ----
            xt_sb = cpool.tile([128, 4, DC, 512], BF16, name="xt_sb")
            warm_sb = cpool.tile([128, 512], BF16, name="warm_sb")
            wq_sb = cpool.tile([128, HC, DC, 128], BF16, name="wq_sb")
            wk_sb = cpool.tile([128, HC, DC, 128], BF16, name="wk_sb")
            wv_sb = cpool.tile([128, DC, DIM], BF16, name="wv_sb")
            wff_sb = cpool.tile([128, HC, DIM], BF16, name="wff_sb")
            qt_sb = cpool.tile([128, HC, T], BF16, name="qt_sb")
            kt_sb = cpool.tile([128, HC, T], BF16, name="kt_sb")
            v_sb = cpool.tile([128, NKC, DIM], BF16, name="v_sb")
            at_sb = cpool.tile([128, HC, T], BF16, name="at_sb")
            bq_sb = cpool.tile([128, HC], F32, name="bq_sb")
            bk_sb = cpool.tile([128, HC], F32, name="bk_sb")
            bffe_sb = cpool.tile([128, DC], F32, name="bffe_sb")
            mv_sb = cpool.tile([128, NKC], F32, name="mv_sb")
            mbc_sb = cpool.tile([128, NKC, 64], BF16, name="mbc_sb")

            # ---- load inputs: batched descriptors (one per tensor slice),
            # split across the two HW DGE queues (sync=SP, scalar=ACT; the
            # ACT queue is idle during the load phase). First Q-proj group
            # needs bq + wq[hc0] + xt quarter 0 only.
            xt_v = xt_d.ap()
            wq_v = wq_d.ap()
            wk_v = wk_d.ap()
            wv_v = wv_d.ap()
            wff_v = wff_d.ap()
            # PE warm-up: ~4us of dummy matmuls on a memset tile during the
            # DMA-load head, so HAM reaches K=8/8 before real matmuls start
            # (else the first ~3.4us of real MMs run at 1.2 GHz).
            nc.gpsimd.memset(warm_sb[:], 0.0)
            warm_ps = mm_pool.tile([128, 512], F32, tag="mm", name="warm_ps")
            for _ in range(13):
                nc.tensor.matmul(
                    warm_ps[:], warm_sb[:, 0:128], warm_sb[:], start=True, stop=True
                )

            # xt-q0 first as one fat descriptor (6KB/partition elements) on
            # sync; wq per-hc on scalar. Both land ~13us, right as the 13
            # warmup matmuls end -> dense warm start, no HAM re-throttle.
            nc.sync.dma_start(out=xt_sb[:, 0], in_=xt_v[:, 0])
            nc.scalar.dma_start(out=wq_sb[:, 0], in_=wq_v[:, 0])
            nc.scalar.dma_start(out=wq_sb[:, 1], in_=wq_v[:, 1])
            nc.sync.dma_start(out=bq_sb[:], in_=bq_d.ap())
            nc.scalar.dma_start(out=wq_sb[:, 2:HC], in_=wq_v[:, 2:HC])
            nc.sync.dma_start(out=bk_sb[:], in_=bk_d.ap())
            nc.sync.dma_start(out=mv_sb[:], in_=mv_d.ap())
            nc.scalar.dma_start(out=wk_sb[:], in_=wk_v[:])
            nc.sync.dma_start(out=wv_sb[:], in_=wv_v[:])
            nc.scalar.dma_start(out=xt_sb[:, 1], in_=xt_v[:, 1])
            # needed only from the attention/FF stages (~25us+)
            nc.sync.dma_start(
                out=mbc_sb[:], in_=mbc_d.ap().rearrange("p (c o) -> p c o", o=64)
            )
            nc.sync.dma_start(out=bffe_sb[:], in_=bffe_d.ap())
            nc.scalar.dma_start(out=xt_sb[:, 2], in_=xt_v[:, 2])
            nc.sync.dma_start(out=xt_sb[:, 3], in_=xt_v[:, 3])
            nc.scalar.dma_start(out=wff_sb[:], in_=wff_v[:])

            def proj_qk(w_sb, b_sb, o_sb, tt):
                # one token-quarter of a Q^T/K^T projection: out [hd, 512].
                # PSUM->SBUF bias copies alternate DVE/ACT so neither queue
                # gates the mm_pool slot recycle during projection phases.
                for hc in range(HC):
                    ps = mm_pool.tile([128, 512], F32, tag="mm", name="ps")
                    for dc in range(DC):
                        nc.tensor.matmul(
                            ps[:],
                            w_sb[:, hc, dc],
                            xt_sb[:, tt, dc],
                            start=(dc == 0),
                            stop=(dc == DC - 1),
                        )
                    if hc % 2 == 0:
                        nc.scalar.activation(
                            o_sb[:, hc, tt * 512:(tt + 1) * 512], ps[:],
                            Ident, bias=b_sb[:, hc:hc + 1], scale=1.0,
                        )
                    else:
                        nc.vector.tensor_scalar(
                            out=o_sb[:, hc, tt * 512:(tt + 1) * 512],
                            in0=ps[:],
                            scalar1=b_sb[:, hc:hc + 1],
                            scalar2=None,
                            op0=ADD,
                        )

            def proj_v(kc):
                # V token-chunk [128 tokens, 768], mask folded in.
                # dc outer / half inner so consecutive matmuls share lhsT.
                tt, c0 = divmod(kc, 4)
                c0 *= 128
                ps = [
                    mm_pool.tile([128, 384], F32, tag="mm", name="ps"),
                    mm_pool.tile([128, 384], F32, tag="mm", name="ps"),
                ]
                for dc in range(DC):
                    for half in range(2):
                        nc.tensor.matmul(
                            ps[half][:],
                            xt_sb[:, tt, dc, c0:c0 + 128],
                            wv_sb[:, dc, half * 384:(half + 1) * 384],
                            start=(dc == 0),
                            stop=(dc == DC - 1),
                        )
                for half in range(2):
                    nc.vector.tensor_scalar(
                        out=v_sb[:, kc, half * 384:(half + 1) * 384],
                        in0=ps[half][:],
                        scalar1=mv_sb[:, kc:kc + 1],
                        scalar2=None,
                        op0=MULT,
                    )

            # ets_by_block[blk] -> list of 6 merged exp tiles [128, 2par,
            # 2kc, 256] bf16
            ets_by_block = {}

            def scores_exp(blk):
                # phase A: scores (row-tiled 64x128, T0/T8 par-pairs pack),
                # both parities of a head pair in one 2-bank PSUM tile, then
                # ONE exp ACTIVATE per head pair over [128, 1024].
                q0 = blk * 256
                ets = []
                for hp in range(H // 2):
                    sp = s_pool.tile([128, 2, 2, 256], F32, tag="s", name="sp")
                    for kc in range(2):
                        k0 = q0 + kc * 128
                        for par in range(2):
                            hr = par * 64
                            nc.tensor.matmul(
                                sp[:, par, kc],
                                kt_sb[hr:hr + 64, hp, k0:k0 + 128],
                                qt_sb[hr:hr + 64, hp, q0:q0 + 256],
                                start=True, stop=True,
                            )
                    et = et_pool.tile([128, 2, 2, 256], BF16, tag="et", name="et")
                    nc.scalar.activation(et[:], sp[:], Exp, bias=0.0, scale=0.125)
                    ets.append(et)
                ets_by_block[blk] = ets

            def avdp(blk):
                # phase B: av (cols 0:256) + denominator (cols 256:512) in
                # one bank, col-tiled 128x64 T0/T1; kc outer / par inner so
                # the (par0, par1) col-tile pairs issue back-to-back and
                # execute concurrently. Emitted one block behind scores_exp
                # so every exp tile is ready -> solid col-mode burst.
                q0 = blk * 256
                ets = ets_by_block.pop(blk)
                for hp in range(H // 2):
                    et = ets[hp]
                    ad = ad_pool.tile([128, 512], F32, tag="ad", name="ad")
                    for kc in range(2):
                        tkc = blk * 2 + kc
                        for par in range(2):
                            hr = par * 64
                            h = 2 * hp + par
                            nc.tensor.matmul(
                                ad[hr:hr + 64, 0:256],
                                v_sb[:, tkc, h * 64:(h + 1) * 64],
                                et[:, par, kc],
                                start=(kc == 0), stop=(kc == 1),
                            )
                    for kc in range(2):
                        tkc = blk * 2 + kc
                        for par in range(2):
                            hr = par * 64
                            nc.tensor.matmul(
                                ad[hr:hr + 64, 256:512],
                                mbc_sb[:, tkc],
                                et[:, par, kc],
                                start=(kc == 0), stop=(kc == 1),
                            )
                    rc = nrm_pool.tile([128, 256], F32, tag="rc", name="rc")
                    nc.vector.reciprocal_approx_fast(rc[:], ad[:, 256:512])
                    nc.vector.tensor_mul(
                        at_sb[:, hp, q0:q0 + 256], ad[:, 0:256], rc[:]
                    )

            def ff(tt):
                for oc in range(DC):
                    ps = mm_pool.tile([128, 512], F32, tag="mm", name="ps")
                    for hc in range(HC):
                        nc.tensor.matmul(
                            ps[:],
                            wff_sb[:, hc, oc * 128:(oc + 1) * 128],
                            at_sb[:, hc, tt * 512:(tt + 1) * 512],
                            start=(hc == 0),
                            stop=(hc == HC - 1),
                        )
                    ob = ob_pool.tile([128, 512], BF16, tag="ob", name="ob")
                    nc.scalar.activation(
                        ob[:], ps[:], Ident, bias=bffe_sb[:, oc:oc + 1], scale=1.0
                    )
                    nc.sync.dma_start(
                        out=out_d.ap()[oc * 128:(oc + 1) * 128,
                                       tt * 512:(tt + 1) * 512],
                        in_=ob[:],
                    )

            # ---- emission: block-pipelined attention. scores_exp(b) runs
            # one block ahead of avdp(b-1) so avdp never waits on ACT and
            # each phase runs as a solid same-tiling-mode burst. ff(u-1)
            # lands between projections and attention as PE filler during
            # the ACT-bound stretches.
            for u in range(4):
                proj_qk(wq_sb, bq_sb, qt_sb, u)
                proj_qk(wk_sb, bk_sb, kt_sb, u)
                for kc in range(4 * u, 4 * u + 4):
                    proj_v(kc)
                if u == 0:
                    scores_exp(0)
                    scores_exp(1)
                    avdp(0)
                else:
                    scores_exp(2 * u)
                    avdp(2 * u - 1)
                    scores_exp(2 * u + 1)
                    avdp(2 * u)
                    # ff(u-1) emitted after attention: ready-but-lower-
                    # priority PE filler for the ACT-bound stretches, and
                    # its ACT bias copies queue behind this quarter's exps
                    ff(u - 1)
            avdp(7)
            ff(3)

    nc.finalize()
    return nc


def _get_nc():
    if "nc" not in _nc_cache:
        _nc_cache["nc"] = _build_nc()
    return _nc_cache["nc"]


def _prep_in_maps(X, mask, Wq, bq, Wk, bk, Wv, bv, Wff, bff):
    bf = ml_dtypes.bfloat16

    def wprep(w):  # [DIM, DIM] -> partition-major [128, DC, DIM]
        return np.ascontiguousarray(
            w.astype(bf).reshape(DC, 128, DIM).transpose(1, 0, 2))

    def wprep_hc(w):  # [DIM, DIM] -> hc-major [128, HC, DC, 128]
        return np.ascontiguousarray(
            w.astype(bf).reshape(DC, 128, HC, 128).transpose(1, 2, 0, 3))

    wq_b, wk_b = wprep_hc(Wq), wprep_hc(Wk)
    wv_b, wff_b = wprep(Wv), wprep(Wff)
    # per-partition bias layouts: [128, nchunks] with col = chunk
    bq_t = np.ascontiguousarray(bq.astype(np.float32).reshape(HC, 128).T)
    bk_t = np.ascontiguousarray(bk.astype(np.float32).reshape(HC, 128).T)
    bffe = (bff.astype(np.float64)
            + bv.astype(np.float64) @ Wff.astype(np.float64)).astype(np.float32)
    bffe_t = np.ascontiguousarray(bffe.reshape(DC, 128).T)

    in_maps = []
    for c in range(NCORES):
        b, s0 = divmod(c, 2)
        s0 *= T
        # partition-major, quarter-major X^T: [128, 4, DC, 512]
        xt = np.ascontiguousarray(
            X[b, s0:s0 + T, :].astype(bf)
            .reshape(4, 512, DC, 128).transpose(3, 0, 2, 1))
        mvalid = (mask[b, s0:s0 + T] > 0).astype(np.float32)  # [T] 0/1
        mv_t = np.ascontiguousarray(mvalid.reshape(NKC, 128).T)  # [128, NKC]
        mbc = np.ascontiguousarray(
            np.broadcast_to(mv_t[:, :, None], (128, NKC, 64))
            .reshape(128, NKC * 64).astype(bf))
        in_maps.append({
            "xt": xt, "wq": wq_b, "wk": wk_b, "wv": wv_b, "wff": wff_b,
            "bq": bq_t, "bk": bk_t, "bffe": bffe_t,
            "mv": mv_t, "mbc": mbc,
        })
    return in_maps


def _assemble(results):
    out = np.empty((B, S, DIM), np.float32)
    for c in range(NCORES):
        b, s0 = divmod(c, 2)
        s0 *= T
        out[b, s0:s0 + T, :] = results[c]["out"].T.astype(np.float32)
    return out


def run(trace=False, **inputs):
    nc = _get_nc()
    in_maps = _prep_in_maps(**inputs)
    res = run_bass_kernel_spmd(
        nc, in_maps, core_ids=list(range(NCORES)), trace=trace
    )
    return _assemble(res.results), res


def kernel(**inputs) -> np.ndarray:
    out, _ = run(trace=False, **inputs)
    return out

